# revision 40
# baseline (speedup 1.0000x reference)
"""Correlation3D Trainium2 kernel v3 (8 NeuronCores, SPMD).

Sharding: core c -> batch c//4, query rows [(c%4)*2048, (c%4)*2048+2048).

vs v2: top-16 selection uses packed values (chunk-global index stored in the
low 13 mantissa bits of the pooled per-chunk top-8 distances) so the merge is
3 scans + 2 bitwise ops instead of 16 match-accumulate passes; l0 scans in 4
2048-wide chunks (was 8x1024); the MLP stacks the 4 pyramid levels on PSUM
partitions ([64,512] pre / one mlp2 matmul per chunk / one shared xyz1-term
matmul), quartering activation-op count and cutting ~12k PE columns per tile.
"""

import numpy as np
import ml_dtypes
from contextlib import ExitStack

import concourse.bass as bass
import concourse.tile as tile
from concourse import bacc, mybir, library_config
from concourse.bass_utils import run_bass_kernel_spmd

bf16 = ml_dtypes.bfloat16
FP = mybir.dt.float32
BF = mybir.dt.bfloat16
U32 = mybir.dt.uint32
I16 = mybir.dt.int16
AF = mybir.ActivationFunctionType
ALU = mybir.AluOpType

B, N1, CIN, C = 2, 8192, 128, 64
NS = [8192, 2048, 512, 128]
KC, KP = 16, 3
NCORES = 8
RPC = N1 * B // NCORES  # 2048
NTILES = RPC // 128     # 16
NEG = -3.0e38
PRIO = 500

MASK_VAL = 0xFFFFE000
MASK_IDX = 0x00001FFF

TERMS = [(0, 0), (0, 1), (1, 0), (0, 2), (2, 0), (1, 1)]


def _split3(x):
    x = np.asarray(x, np.float64)
    t0 = x.astype(bf16).astype(np.float64)
    t1 = (x - t0).astype(bf16).astype(np.float64)
    t2 = (x - t0 - t1).astype(bf16).astype(np.float64)
    return [t.astype(np.float32) for t in (t0, t1, t2)]


def _dist_moving(xyz):
    p2 = (np.asarray(xyz, np.float64) ** 2).sum(0)
    ps = [_split3(xyz[d]) for d in range(3)]
    rows = []
    for d in range(3):
        for (_, j) in TERMS:
            rows.append(ps[d][j])
    rows += _split3(p2)
    return np.stack(rows).astype(bf16)  # [21, N]


def _dist_stationary(xyz):
    qs = [_split3(xyz[d]) for d in range(3)]
    rows = []
    for d in range(3):
        for (i, _) in TERMS:
            rows.append(2.0 * qs[d][i])
    one = -np.ones(xyz.shape[1], np.float32)
    rows += [one, one, one]
    return np.stack(rows).astype(bf16)  # [21, N]


def _a1_stat(w1, b1):
    w1h = w1[:, :3].astype(bf16).astype(np.float32)
    w1l = (w1[:, :3] - w1h).astype(bf16).astype(np.float32)
    b1h = b1.astype(bf16).astype(np.float32)
    b1l = (b1 - b1h).astype(bf16).astype(np.float32)
    stat = np.zeros((11, 16), np.float32)
    for d in range(3):
        stat[d] = -w1h[:, d]
        stat[3 + d] = -w1h[:, d]
        stat[6 + d] = -w1l[:, d]
    stat[9] = b1h
    stat[10] = b1l
    return stat.astype(bf16)


def _a1_moving(xyz1own):
    qs = [_split3(xyz1own[d]) for d in range(3)]
    n = xyz1own.shape[1]
    rows = [qs[0][0], qs[1][0], qs[2][0]]
    rows += [(qs[d][1] + qs[d][2]).astype(bf16).astype(np.float32) for d in range(3)]
    rows += [qs[0][0], qs[1][0], qs[2][0]]
    one = np.ones(n, np.float32)
    rows += [one, one]
    return np.stack(rows).astype(bf16)  # [11, N]


def _split2T(w):
    h = w.astype(bf16).astype(np.float32)
    l = (w - h).astype(bf16)
    return np.concatenate([h.astype(bf16).T, l.T], axis=0)  # [2K, M]


def _w2_stack(w2):
    # [128, 64]: h1dup partition layout is [l hi(16) | l lo(16)] per level l
    # (32-partition groups); columns are the dense (l, o) cost layout.
    w2h = w2.astype(bf16).astype(np.float32)
    w2l = (w2 - w2h).astype(np.float32)
    stk = np.zeros((128, 64), np.float32)
    for l in range(4):
        stk[32 * l:32 * l + 16, 16 * l:16 * l + 16] = w2h.T
        stk[32 * l + 16:32 * l + 32, 16 * l:16 * l + 16] = w2l.T
    return stk.astype(bf16)


def _knn3(inp, qry):
    q2 = (qry.astype(np.float32) ** 2).sum(0)[:, None]
    i2 = (inp.astype(np.float32) ** 2).sum(0)[None, :]
    cross = qry.T.astype(np.float32) @ inp.astype(np.float32)
    d = q2 + i2 - 2.0 * cross
    return np.argpartition(d, KP, axis=1)[:, :KP]


def build_host_inputs(inputs):
    xyz1 = np.asarray(inputs['xyz1'], np.float32)
    lv = [np.asarray(inputs[f'xyz2_{i}'], np.float32) for i in range(4)]
    feat1 = np.asarray(inputs['feat1'], np.float32)
    feat2 = np.asarray(inputs['feat2'], np.float32)
    w1 = np.asarray(inputs['w1'], np.float32); b1 = np.asarray(inputs['b1'], np.float32)
    w2 = np.asarray(inputs['w2'], np.float32); b2 = np.asarray(inputs['b2'], np.float32)
    wm = np.asarray(inputs['wm'], np.float32); bm = np.asarray(inputs['bm'], np.float32)

    per_batch = []
    for b in range(B):
        pb = {}
        f2eff = [feat2[b]]
        for l in range(1, 4):
            idx3 = _knn3(lv[l - 1][b], lv[l][b])
            f2eff.append(f2eff[l - 1][:, idx3].mean(axis=2, dtype=np.float64)
                         .astype(np.float32))
        for l in range(4):
            pb[f'mov{l}'] = np.ascontiguousarray(_dist_moving(lv[l][b]))
            u = w1[:, :3].astype(np.float64) @ lv[l][b].astype(np.float64)  # [16, n]
            uh = u.astype(bf16)
            ul = (u - uh.astype(np.float64)).astype(bf16)
            td = np.zeros((NS[l], 256), bf16)
            td[:, 0:128] = f2eff[l].T.astype(bf16)
            td[:, 128:144] = uh.T
            td[:, 144:160] = ul.T
            pb[f'td{l}'] = np.ascontiguousarray(td)
        per_batch.append(pb)

    a1s = _a1_stat(w1, b1).astype(np.float32)
    a1z = np.zeros((11, 64), np.float32)
    a1z[:, 0:16] = a1s
    a1z[:, 32:48] = a1s  # levels at partition bases 0 and 32; 16-31/48-63 junk
    common = {
        'a1stat2': np.ascontiguousarray(a1z.astype(bf16)),  # [11, 64]
        'w14g': np.ascontiguousarray(
            np.tile(w1[:, 3].astype(np.float32)[None, :], (128, 1)).astype(bf16)),
        'identb': np.concatenate([np.eye(16), np.eye(16)],
                                 axis=0).astype(bf16),  # [32, 16]
        'w2stk': np.ascontiguousarray(_w2_stack(w2)),            # [128, 64]
        'wmstk': np.ascontiguousarray(_split2T(wm)),             # [128, 64]
        'b2c4': np.ascontiguousarray(np.tile(b2, 4).reshape(64, 1)
                                     .astype(np.float32)),
        'bmc': np.ascontiguousarray(bm.reshape(64, 1)),
        'ident': np.eye(128, dtype=np.float32),
        'coff0': np.ascontiguousarray(np.tile(
            (np.arange(64, dtype=np.uint32) // 8 * 1024), (128, 1))),
        'coff1': np.ascontiguousarray(np.tile(
            (np.arange(32, dtype=np.uint32) // 8 * 512), (128, 1))),
        'repf': np.ascontiguousarray(
            np.tile(np.eye(16, dtype=np.float32), (1, 8))),  # [16, 128]
    }

    in_maps = []
    for core in range(NCORES):
        b = core // 4
        r0 = (core % 4) * RPC
        sl = slice(r0, r0 + RPC)
        m = dict(common)
        m.update(per_batch[b])
        m['qstat'] = np.ascontiguousarray(_dist_stationary(xyz1[b][:, sl]))
        m['a1mov'] = np.ascontiguousarray(_a1_moving(xyz1[b][:, sl]))
        m['f1rep'] = np.ascontiguousarray(
            np.repeat(feat1[b][:, sl] / np.float32(CIN), KC, axis=1).astype(bf16))
        in_maps.append(m)
    return in_maps


# ---------------------------------------------------------------------------

def build_program():
    nc = bacc.Bacc("TRN2", target_bir_lowering=False, debug=False,
                   num_devices=NCORES)

    def din(name, shape, dt):
        return nc.dram_tensor(name, list(shape), dt, kind="ExternalInput").ap()

    qstatD = din('qstat', (21, RPC), BF)
    a1movD = din('a1mov', (11, RPC), BF)
    f1repD = din('f1rep', (128, RPC * KC), BF)
    movD = [din(f'mov{l}', (21, NS[l]), BF) for l in range(4)]
    tdD = [din(f'td{l}', (NS[l], 256), BF) for l in range(4)]
    a1stat2D = din('a1stat2', (11, 64), BF)
    w14gD = din('w14g', (128, 16), BF)
    identbD = din('identb', (32, 16), BF)
    w2stkD = din('w2stk', (128, 64), BF)
    wmstkD = din('wmstk', (128, 64), BF)
    b2c4D = din('b2c4', (64, 1), FP)
    bmcD = din('bmc', (64, 1), FP)
    identD = din('ident', (128, 128), FP)
    coff0D = din('coff0', (128, 64), U32)
    coff1D = din('coff1', (128, 32), U32)
    repfD = din('repf', (16, 128), FP)

    outD = nc.dram_tensor('out', [C, RPC], FP, kind="ExternalOutput").ap()

    with tile.TileContext(nc) as tc, ExitStack() as ctx:
        nc.gpsimd.load_library(library_config.mlp)
        cpool = ctx.enter_context(tc.tile_pool(name="const", bufs=1))
        sel = ctx.enter_context(tc.tile_pool(name="sel", bufs=1))
        gat = ctx.enter_context(tc.tile_pool(name="gat", bufs=1))
        mlp = ctx.enter_context(tc.tile_pool(name="mlp", bufs=1))
        psd = ctx.enter_context(tc.tile_pool(name="psd", bufs=2, space="PSUM"))
        psm = ctx.enter_context(tc.tile_pool(name="psm", bufs=2, space="PSUM"))
        psacc = ctx.enter_context(tc.tile_pool(name="psacc", bufs=1, space="PSUM"))
        pstr = ctx.enter_context(tc.tile_pool(name="pstr", bufs=1, space="PSUM"))

        def load(pool, ap, tag):
            t = pool.tile(list(ap.shape), ap.dtype, tag=tag)
            nc.sync.dma_start(t[:], ap[:])
            return t

        qstat = load(cpool, qstatD, 'qstat')
        # split mov0's 344KB load so tile 0's first fills start sooner
        mov0t = cpool.tile([21, NS[0]], BF, tag='mov0')
        for _pc in range(8):
            nc.sync.dma_start(mov0t[:, 1024 * _pc:1024 * (_pc + 1)],
                              movD[0][:, 1024 * _pc:1024 * (_pc + 1)])
        mov = [mov0t] + [load(cpool, movD[l], f'mov{l}') for l in (1, 2, 3)]
        ident = load(cpool, identD, 'ident')
        coff0 = load(cpool, coff0D, 'coff0')
        coff1 = load(cpool, coff1D, 'coff1')
        identb = load(cpool, identbD, 'identb')
        repf = load(cpool, repfD, 'repf')
        a1mov = load(cpool, a1movD, 'a1mov')
        a1stat2 = load(cpool, a1stat2D, 'a1stat2')
        w14g = load(cpool, w14gD, 'w14g')
        w2stk = load(cpool, w2stkD, 'w2stk')
        wmstk = load(cpool, wmstkD, 'wmstk')
        b2c4 = load(cpool, b2c4D, 'b2c4')
        bmc = load(cpool, bmcD, 'bmc')

        # ------------- helpers ---------------------------------------------

        def pack_merge(t, l, poolv, i8p, npool):
            """poolv [128, npool] fp32 per-chunk top-8 maxima; i8p [128, npool]
            u32 chunk-GLOBAL indices (<8192). Packs idx into the low 13 bits,
            takes top-16 of the pool, unpacks -> g16 [128, 16] fp32."""
            pu = poolv[:].bitcast(U32)
            nc.vector.tensor_scalar(pu, pu, MASK_VAL, None, ALU.bitwise_and)
            nc.vector.tensor_tensor(pu, pu, i8p[:], ALU.bitwise_or)
            w16 = sel.tile([128, 16], FP, tag="w16", bufs=2)
            nc.vector.max(w16[:, 0:8], poolv[:])
            poolb = sel.tile([128, npool], FP, tag=f"poolb{l}", bufs=1,
                             name=f"poolb{l}")
            nc.vector.match_replace(poolb[:], w16[:, 0:8], poolv[:], NEG)
            nc.vector.max(w16[:, 8:16], poolb[:])
            g16u = sel.tile([128, 16], U32, tag="g16u", bufs=2)
            nc.vector.tensor_scalar(g16u[:], w16[:].bitcast(U32), MASK_IDX,
                                    None, ALU.bitwise_and)
            g16 = sel.tile([128, 16], FP, tag=f"g16_{l}", bufs=2)
            nc.vector.tensor_copy(g16[:], g16u[:])
            return g16

        def exact16(t, l, ps, n):
            w = sel.tile([128, 16], FP, tag="selw", bufs=2)
            i8 = sel.tile([128, 16], U32, tag=f"i8_{l}", bufs=2)
            nc.vector.max(w[:, 0:8], ps[:, 0:n])
            nc.vector.max_index(i8[:, 0:8], w[:, 0:8], ps[:, 0:n])
            nc.vector.match_replace(ps[:, 0:n], w[:, 0:8], ps[:, 0:n], NEG)
            nc.vector.max(w[:, 8:16], ps[:, 0:n])
            nc.vector.max_index(i8[:, 8:16], w[:, 8:16], ps[:, 0:n])
            g16 = sel.tile([128, 16], FP, tag=f"g16_{l}", bufs=2)
            nc.vector.tensor_copy(g16[:], i8[:])
            return g16

        def wrap_gather(t, l, g16):
            """g16 [128,16] -> wrapped+replicated idx -> int16 -> dma_gather."""
            ps2 = pstr.tile([128, 128], FP, tag="trps", bufs=1)
            nc.tensor.transpose(ps2[0:16, :], g16[:], ident[:])
            g16f = sel.tile([16, 128], FP, tag="g16f", bufs=2)
            nc.scalar.activation(g16f[:], ps2[0:16, :], AF.Copy)
            nc.tensor.matmul(ps2[:], repf[:], g16f[:], start=True, stop=True)
            wi = sel.tile([128, 128], I16, tag=f"wi{l}", bufs=2)
            nc.scalar.activation(wi[:], ps2[:], AF.Copy)
            gt = gat.tile([128, 2, 2048], BF, tag=f"gt{l}", bufs=3)
            nc.gpsimd.dma_gather(gt[:], tdD[l][:], wi[:],
                                 num_idxs=RPC, num_idxs_reg=RPC,
                                 elem_size=256, transpose=True)
            return gt

        def emit_Gs(pv, eng=None):
            eng = eng or nc.gpsimd
            Gs = []
            for l in range(4):
                f2g = pv['gts'][l][:, 0:1, :].rearrange("p a b -> p (a b)")
                G = mlp.tile([128, 2048], BF, tag=f"G{l}", bufs=2)
                eng.tensor_mul(G[:], f2g, pv['f1t'][:])
                Gs.append(G)
            pv['Gs'] = Gs

        def build_pieces(pv):
            t, gts = pv['t'], pv['gts']
            Gs = pv['Gs']
            a1v = a1mov[:, 128 * t:128 * (t + 1)].unsqueeze(2) \
                .broadcast_to([11, 128, 16])
            h1dup = mlp.tile([128, 2048], BF, tag="h1dup", bufs=2)
            h2dup = mlp.tile([128, 2048], BF, tag="h2dup", bufs=2)

            def uhl(l):
                return gts[l][0:32, 1:2, :].rearrange("p a b -> p (a b)")

            def mlp1_half(c, half):
                csl = slice(512 * c, 512 * (c + 1))
                qsl = slice(32 * c, 32 * (c + 1))
                pre = psm.tile([64, 512], FP, tag="pre", bufs=2)
                for s in range(2):
                    l = 2 * half + s
                    nc.tensor.matmul(pre[32 * s:32 * s + 16, :], w14g[:],
                                     Gs[l][:, csl], start=True, stop=False,
                                     skip_group_check=True)
                for s in range(2):
                    l = 2 * half + s
                    nc.tensor.matmul(pre[32 * s:32 * s + 16, :], identb[:],
                                     uhl(l)[:, csl], start=False,
                                     stop=False, skip_group_check=True)
                nc.tensor.matmul(pre[:], a1stat2[:], a1v[:, qsl, :],
                                 start=False, stop=True,
                                 skip_group_check=True)
                nc.scalar.activation(h1dup[64 * half:64 * half + 64, csl],
                                     pre[:], AF.Relu)

            def h1_dup():
                for g in range(4):
                    nc.sync.dma_start(h1dup[32 * g + 16:32 * g + 32, :],
                                      h1dup[32 * g:32 * g + 16, :])

            def mlp2_chunk(c):
                csl = slice(512 * c, 512 * (c + 1))
                h2p = psm.tile([64, 512], FP, tag="pre", bufs=2)
                nc.tensor.matmul(h2p[:], w2stk[:], h1dup[:, csl],
                                 start=True, stop=True)
                nc.scalar.activation(h2dup[0:64, csl], h2p[:], AF.Relu,
                                     bias=b2c4[:])

            def h2_dup():
                nc.sync.dma_start(h2dup[64:128, :], h2dup[0:64, :])

            def final_out():
                acc = psacc.tile([64, 128], FP, tag="accps")
                h2v = h2dup[:].rearrange("p (q k) -> p k q", k=KC)
                for k in range(KC):
                    nc.tensor.matmul(acc[:], wmstk[:],
                                     h2v[:, k:k + 1, :].squeeze(1),
                                     start=(k == 0), stop=(k == KC - 1))
                ot = mlp.tile([64, 128], FP, tag="outt", bufs=2)
                nc.scalar.activation(ot[:], acc[:], AF.Relu, bias=bmc[:])
                nc.sync.dma_start(outD[:, 128 * t:128 * (t + 1)], ot[:])

            pieces = [lambda c=c: mlp1_half(c, 0) for c in range(4)]
            pieces += [lambda c=c: mlp1_half(c, 1) for c in range(4)]
            pieces.append(h1_dup)
            pieces += [lambda c=c: mlp2_chunk(c) for c in range(4)]
            pieces.append(h2_dup)
            pieces.append(final_out)
            return pieces

        # ------------- main pipeline ---------------------------------------
        def emit_M(rec):
            """Merges, wraps, gathers and G-multiplies for a scanned tile."""
            t = rec['t']
            gts = rec['gts']
            g16 = pack_merge(t, 0, rec['poolv'], rec['i8p'], 64)
            gts.append(wrap_gather(t, 0, g16))
            g16 = pack_merge(t, 1, rec['poolv1'], rec['i8p1'], 32)
            gts.append(wrap_gather(t, 1, g16))
            gts.append(wrap_gather(t, 2, rec['g16l'][2]))
            gts.append(wrap_gather(t, 3, rec['g16l'][3]))
            emit_Gs(rec)

        pieces = []
        recs = {}
        for t in range(NTILES):
            qs = qstat[:, 128 * t:128 * (t + 1)]
            f1t = mlp.tile([128, 2048], BF, tag="f1t", bufs=2)
            nc.sync.dma_start(f1t[:], f1repD[:, RPC * t:RPC * (t + 1)])
            pieces_n = len(pieces)

            def drain(k):
                for _ in range(k):
                    if pieces:
                        pieces.pop(0)()
            # ---- level 0: 8 chunks x 1024, top-8 each, packed merge -------
            poolv = sel.tile([128, 64], FP, tag="poolv0", bufs=2)
            i8p = sel.tile([128, 64], U32, tag="i8p0", bufs=2)
            for c in range(8):
                ps = psd.tile([128, 1024], FP, tag="dps")
                with tc.high_priority(offset=PRIO):
                    nc.tensor.matmul(ps[:, 0:512], qs,
                                     mov[0][:, 1024 * c:1024 * c + 512],
                                     start=True, stop=True)
                    nc.tensor.matmul(ps[:, 512:1024], qs,
                                     mov[0][:, 1024 * c + 512:1024 * (c + 1)],
                                     start=True, stop=True)
                nc.vector.max(poolv[:, 8 * c:8 * c + 8], ps[:])
                nc.vector.max_index(i8p[:, 8 * c:8 * c + 8],
                                    poolv[:, 8 * c:8 * c + 8], ps[:])
                if c >= 1:
                    drain(1)
            # ---- level 1: 2 fills, 4x512 chunks, packed merge -------------
            poolv1 = sel.tile([128, 32], FP, tag="poolv1", bufs=2)
            i8p1 = sel.tile([128, 32], U32, tag="i8p1", bufs=2)
            for half in range(2):
                ps = psd.tile([128, 1024], FP, tag="dps")
                with tc.high_priority(offset=PRIO):
                    nc.tensor.matmul(ps[:, 0:512], qs,
                                     mov[1][:, 1024 * half:1024 * half + 512],
                                     start=True, stop=True)
                    nc.tensor.matmul(ps[:, 512:1024], qs,
                                     mov[1][:, 1024 * half + 512:1024 * (half + 1)],
                                     start=True, stop=True)
                for cc in range(2):
                    c = 2 * half + cc
                    csl = slice(512 * cc, 512 * (cc + 1))
                    nc.vector.max(poolv1[:, 8 * c:8 * c + 8], ps[:, csl])
                    nc.vector.max_index(i8p1[:, 8 * c:8 * c + 8],
                                        poolv1[:, 8 * c:8 * c + 8], ps[:, csl])
                drain(1)
            # ---- levels 2, 3: fill then exact top-16; wraps as soon as
            # each level's selection is done so gathers+G start early -------
            gts = []
            nc.vector.tensor_tensor(i8p[:], i8p[:], coff0[:], ALU.add)
            g16 = pack_merge(t, 0, poolv, i8p, 64)
            gts.append(wrap_gather(t, 0, g16))
            drain(1)
            nc.vector.tensor_tensor(i8p1[:], i8p1[:], coff1[:], ALU.add)
            g16 = pack_merge(t, 1, poolv1, i8p1, 32)
            gts.append(wrap_gather(t, 1, g16))
            drain(2)
            last = (t == NTILES - 1)
            lastGs = []
            if last:
                # interleave G right behind each wrap so DVE G-multiplies
                # start as soon as each gather lands, and start the mlp1
                # A-half pieces as soon as G0/G1 exist (shortens the tail)
                for li in (0, 1):
                    f2g = gts[li][:, 0:1, :].rearrange("p a b -> p (a b)")
                    Gl = mlp.tile([128, 2048], BF, tag=f"G{li}", bufs=2)
                    nc.vector.tensor_mul(Gl[:], f2g, f1t[:])
                    lastGs.append(Gl)
                while pieces:
                    pieces.pop(0)()
                cur = {'t': t, 'gts': gts, 'f1t': f1t, 'Gs': lastGs}
                pieces = build_pieces(cur)
                drain(4)
            for l in (2, 3):
                n = NS[l]
                ps = psd.tile([128, 1024], FP, tag="dps")
                with tc.high_priority(offset=PRIO):
                    nc.tensor.matmul(ps[:, 0:n], qs, mov[l][:, 0:n],
                                     start=True, stop=True)
                g16 = exact16(t, l, ps, n)
                if not last:
                    drain(2)
                gts.append(wrap_gather(t, l, g16))
                if last:
                    f2g = gts[l][:, 0:1, :].rearrange("p a b -> p (a b)")
                    Gl = mlp.tile([128, 2048], BF, tag=f"G{l}", bufs=2)
                    nc.vector.tensor_mul(Gl[:], f2g, f1t[:])
                    lastGs.append(Gl)
            if not last:
                while pieces:
                    pieces.pop(0)()
                cur = {'t': t, 'gts': gts, 'f1t': f1t}
                emit_Gs(cur)
                pieces = build_pieces(cur)
            recs[t] = cur
        for piece in pieces:
            piece()

    nc.compile()
    return nc


_NC_CACHE = None


def kernel(**inputs):
    global _NC_CACHE
    in_maps = build_host_inputs(inputs)
    if _NC_CACHE is None:
        _NC_CACHE = build_program()
    try:
        res = run_bass_kernel_spmd(_NC_CACHE, in_maps, list(range(NCORES))).results
    except Exception:
        from concourse.bass_interp import MultiCoreSim
        res = []
        for core in range(NCORES):
            sim = MultiCoreSim(build_program(), 1,
                               require_finite=False, require_nnan=False)
            for name, arr in in_maps[core].items():
                sim.cores[0].tensor(name)[:] = arr
            sim.simulate()
            res.append({'out': np.asarray(sim.cores[0].tensor('out')).copy()})
    out = np.zeros((B, C, N1), np.float32)
    for core in range(NCORES):
        b = core // 4
        r0 = (core % 4) * RPC
        out[b][:, r0:r0 + RPC] = res[core]['out']
    return out
